# revision 1
# baseline (speedup 1.0000x reference)
"""MoE (8 routed experts, top-2, + shared expert) on 8 TRN2 NeuronCores.

Strategy: expert-parallel. Host computes the gate (fp32 numpy, exactly
mirroring the reference), gathers each expert's tokens, and core e runs
expert e's SwiGLU (h = silu(x@w1T) * (x@w3T) * cw; y = h_bf16 @ w2T)
over its gathered tokens, plus a 1/8 token-slice of the shared expert.
Host scatters expert outputs back and combines in bf16 expert order.

All tensors fed to the device are pre-arranged on host into
partition-major layouts so every DMA is contiguous per partition:
  activations/weights for matmul lhsT/rhs always have the contraction
  dim chunked as [pi=128, po, free].
"""

import numpy as np
import ml_dtypes

import concourse.mybir as mybir
from concourse import bacc
from concourse.tile import TileContext
from concourse import bass_utils

BF16 = mybir.dt.bfloat16
F32 = mybir.dt.float32

D = 2048          # model dim
I = 1408          # expert inter dim
E = 8             # routed experts
TOPK = 2
N_CORES = 8
DPO = D // 128    # 16 chunks of the model dim
IPO = I // 128    # 11 chunks of the inter dim

_BUILD_CACHE = {}


def _c_blocks(C):
    """Split C columns into equal-ish blocks <= 512, multiples of 128."""
    nb = -(-C // 512)
    per = -(-C // (nb * 128)) * 128
    blocks = []
    off = 0
    while off < C:
        w = min(per, C - off)
        blocks.append((off, w))
        off += w
    return blocks


def _build(C, TS):
    """Build the per-core Bass kernel for routed capacity C and shared
    token-slice TS. Same NEFF runs SPMD on all 8 cores."""
    nc = bacc.Bacc("TRN2", debug=False, enable_asserts=False,
                   num_devices=N_CORES, enable_partition_id=False)

    def din(name, shape, dt=BF16):
        return nc.dram_tensor(name, shape, dt, kind="ExternalInput").ap()

    def dout(name, shape, dt=BF16):
        return nc.dram_tensor(name, shape, dt, kind="ExternalOutput").ap()

    xr = din("xr", [128, DPO, C])            # routed tokens, [d_pi, d_po, c]
    xs = din("xs", [128, DPO, TS])           # shared-expert token slice
    cwb = din("cwb", [128, C], F32)          # combine weights, replicated
    w1t = din("w1t", [IPO, 128, D])          # [i_blk][d_pi][d_po*128+i_c]
    w3t = din("w3t", [IPO, 128, D])
    w2t = din("w2t", [DPO, 128, I])          # [d_blk][i_pi][i_po*128+d_c]
    sw1t = din("sw1t", [IPO, 128, D])
    sw3t = din("sw3t", [IPO, 128, D])
    sw2t = din("sw2t", [DPO, 128, I])
    ye = dout("ye", [128, DPO, C])           # [d_pi, d_po, c]
    zs = dout("zs", [128, DPO, TS])

    Silu = mybir.ActivationFunctionType.Silu

    with TileContext(nc) as tc:
        with tc.tile_pool(name="main", bufs=1) as pool, \
             tc.tile_pool(name="psum", bufs=1, space="PSUM") as pp:
            cw_sb = pool.tile([128, C], F32, tag="cwb", bufs=1, name="cw_sb")

            # routed job first: the second job's startup stream then
            # overlaps the first job's ~113us of phase-B PE work, and the
            # small xs stream doesn't starve routed phase-B weight loads
            jobs = [
                ("r", C, xr, w1t, w3t, w2t, ye, True),
                ("s", TS, xs, sw1t, sw3t, sw2t, zs, False),
            ]
            for jname, CJ, x_d, w1_d, w3_d, w2_d, out_d, scaled in jobs:
                cbs = _c_blocks(CJ)
                x_sb = pool.tile([128, DPO, CJ], BF16, tag=f"x_{jname}",
                                 bufs=1, name=f"x_{jname}")
                # startup: land just enough bytes for the first matmuls
                # (x slice 0 + the first weight chunks) before streaming
                # the rest, so the PE starts ~10us in instead of ~25us
                nc.sync.dma_start(x_sb[:, 0, :], x_d[:, 0, :])
                w13_first = []
                wdr = []
                for wd, wn in ((w1_d, "w1"), (w3_d, "w3")):
                    w_sb = pool.tile([128, DPO, 128], BF16, tag="w13",
                                     bufs=6, name=f"{wn}_{jname}_0")
                    w13_first.append(w_sb)
                    wdr.append(wd[0].rearrange("p (a b) -> p a b", a=DPO))
                for w_sb, wsrc in zip(w13_first, wdr):
                    nc.sync.dma_start(w_sb[:, 0:4, :], wsrc[:, 0:4, :])
                for w_sb, wsrc in zip(w13_first, wdr):
                    nc.sync.dma_start(w_sb[:, 4:, :], wsrc[:, 4:, :])
                for dsl in range(1, DPO):
                    nc.sync.dma_start(x_sb[:, dsl, :], x_d[:, dsl, :])
                if scaled:
                    nc.sync.dma_start(cw_sb[:], cwb[:])
                H = pool.tile([128, IPO, CJ], BF16, tag=f"H_{jname}",
                              bufs=1, name=f"H_{jname}")

                # ---- phase A: H = silu(x@w1T) * (x@w3T) [* cw] ----
                for i in range(IPO):
                    if i == 0:
                        w1_sb, w3_sb = w13_first
                    else:
                        w1_sb = pool.tile([128, DPO, 128], BF16, tag="w13",
                                          bufs=6, name=f"w1_{jname}_{i}")
                        nc.sync.dma_start(
                            w1_sb[:],
                            w1_d[i].rearrange("p (a b) -> p a b", a=DPO))
                        w3_sb = pool.tile([128, DPO, 128], BF16, tag="w13",
                                          bufs=6, name=f"w3_{jname}_{i}")
                        nc.sync.dma_start(
                            w3_sb[:],
                            w3_d[i].rearrange("p (a b) -> p a b", a=DPO))
                    p1s = []
                    p3s = []
                    for bi, (off, w) in enumerate(cbs):
                        p1s.append(pp.tile([128, w], F32, tag="ps", bufs=8,
                                           name=f"p1_{jname}_{i}_{bi}"))
                        p3s.append(pp.tile([128, w], F32, tag="ps", bufs=8,
                                           name=f"p3_{jname}_{i}_{bi}"))
                    for d in range(DPO):
                        for bi, (off, w) in enumerate(cbs):
                            nc.tensor.matmul(
                                p1s[bi][:], w1_sb[:, d, :],
                                x_sb[:, d, off:off + w],
                                start=(d == 0), stop=(d == DPO - 1))
                        for bi, (off, w) in enumerate(cbs):
                            nc.tensor.matmul(
                                p3s[bi][:], w3_sb[:, d, :],
                                x_sb[:, d, off:off + w],
                                start=(d == 0), stop=(d == DPO - 1))
                    for bi, (off, w) in enumerate(cbs):
                        s_t = pool.tile([128, w], F32, tag="act1", bufs=6,
                                        name=f"s_{jname}_{i}_{bi}")
                        nc.scalar.activation(s_t[:], p1s[bi][:], Silu)
                        if scaled:
                            t_t = pool.tile([128, w], F32, tag="act2", bufs=6,
                                            name=f"t_{jname}_{i}_{bi}")
                            nc.vector.tensor_mul(t_t[:], p3s[bi][:],
                                                 cw_sb[:, off:off + w])
                            nc.vector.tensor_mul(H[:, i, off:off + w],
                                                 s_t[:], t_t[:])
                        else:
                            nc.vector.tensor_mul(H[:, i, off:off + w],
                                                 s_t[:], p3s[bi][:])

                # ---- phase B: out = H @ w2T ----
                for do in range(DPO):
                    w2_sb = pool.tile([128, IPO, 128], BF16, tag="w2",
                                      bufs=5, name=f"w2_{jname}_{do}")
                    nc.sync.dma_start(
                        w2_sb[:], w2_d[do].rearrange("p (a b) -> p a b", a=IPO))
                    pys = []
                    for bi, (off, w) in enumerate(cbs):
                        pys.append(pp.tile([128, w], F32, tag="ps", bufs=8,
                                           name=f"py_{jname}_{do}_{bi}"))
                    for i in range(IPO):
                        for bi, (off, w) in enumerate(cbs):
                            nc.tensor.matmul(
                                pys[bi][:], w2_sb[:, i, :],
                                H[:, i, off:off + w],
                                start=(i == 0), stop=(i == IPO - 1))
                    for bi, (off, w) in enumerate(cbs):
                        y_t = pool.tile([128, w], BF16, tag="yo", bufs=8,
                                        name=f"y_{jname}_{do}_{bi}")
                        nc.vector.tensor_copy(y_t[:], pys[bi][:])
                        nc.sync.dma_start(out_d[:, do, off:off + w], y_t[:])

    nc.finalize()
    return nc


def _get_kernel(C, TS):
    key = (C, TS)
    if key not in _BUILD_CACHE:
        _BUILD_CACHE[key] = _build(C, TS)
    return _BUILD_CACHE[key]


def _pm(a, po):
    """[N, po*128] -> partition-major [128, po, N] contiguous."""
    n = a.shape[0]
    return np.ascontiguousarray(
        a.T.reshape(po, 128, n).transpose(1, 0, 2))


def kernel(x, gate_w, gate_b, w1, w2, w3, sw1, sw2, sw3):
    bf16 = ml_dtypes.bfloat16
    x = np.asarray(x)
    gate_w = np.asarray(gate_w, dtype=np.float32)
    gate_b = np.asarray(gate_b, dtype=np.float32)
    w1 = np.asarray(w1)
    w2 = np.asarray(w2)
    w3 = np.asarray(w3)
    sw1 = np.asarray(sw1)
    sw2 = np.asarray(sw2)
    sw3 = np.asarray(sw3)

    B, S, Dx = x.shape
    assert Dx == D
    T = B * S
    TS = T // N_CORES
    xt = x.reshape(T, D)

    # ---- gate (fp32, mirrors reference: sqrt(softplus), top-2 on biased) ----
    xf = xt.astype(np.float32)
    logits = xf @ gate_w.T
    scores = np.sqrt(np.log1p(np.exp(-np.abs(logits)))
                     + np.maximum(logits, 0.0))
    biased = scores + gate_b
    idx = np.argsort(-biased, axis=1, kind="stable")[:, :TOPK]
    cw = np.zeros((T, E), dtype=np.float32)
    np.put_along_axis(cw, idx, np.take_along_axis(scores, idx, axis=1), axis=1)

    sel = np.zeros((T, E), dtype=bool)
    np.put_along_axis(sel, idx, True, axis=1)
    tok_lists = [np.nonzero(sel[:, e])[0] for e in range(E)]
    counts = np.array([len(t) for t in tok_lists])
    C = max(256, int(-(-counts.max() // 128) * 128))

    nc = _get_kernel(C, TS)

    # ---- per-core input prep ----
    # weight transforms: lhsT layouts, block-major so DMAs are contiguous
    def wA_layout(wm):  # [I, D] -> [IPO, 128, D]; [ib,pi,po*128+ic]
        return np.ascontiguousarray(
            wm.T.reshape(DPO, 128, IPO, 128).transpose(2, 1, 0, 3)
        ).reshape(IPO, 128, D)

    def wB_layout(wm):  # [D, I] -> [DPO, 128, I]; [db,pi,po*128+dc]
        return np.ascontiguousarray(
            wm.T.reshape(IPO, 128, DPO, 128).transpose(2, 1, 0, 3)
        ).reshape(DPO, 128, I)

    sw1t = wA_layout(sw1)
    sw3t = wA_layout(sw3)
    sw2t = wB_layout(sw2)

    in_maps = []
    for e in range(E):
        toks = tok_lists[e]
        cnt = len(toks)
        xg = np.zeros((C, D), dtype=bf16)
        xg[:cnt] = xt[toks]
        cwe = np.zeros((C,), dtype=np.float32)
        cwe[:cnt] = cw[toks, e]
        xs_slice = xt[e * TS:(e + 1) * TS]
        in_maps.append({
            "xr": _pm(xg, DPO),
            "xs": _pm(xs_slice, DPO),
            "cwb": np.ascontiguousarray(
                np.broadcast_to(cwe[None, :], (128, C))),
            "w1t": wA_layout(w1[e]),
            "w3t": wA_layout(w3[e]),
            "w2t": wB_layout(w2[e]),
            "sw1t": sw1t,
            "sw3t": sw3t,
            "sw2t": sw2t,
        })

    res = bass_utils.run_bass_kernel_spmd(
        nc, in_maps, core_ids=list(range(N_CORES)))
    global LAST_RESULT
    LAST_RESULT = res

    # ---- unshard + combine (bf16, reference addition order) ----
    y = np.zeros((T, D), dtype=bf16)
    for e in range(E):
        toks = tok_lists[e]
        cnt = len(toks)
        ye = res.results[e]["ye"]                       # [128, DPO, C]
        ye_tok = ye.transpose(2, 1, 0).reshape(C, D)    # [c, d]
        y[toks] = y[toks] + ye_tok[:cnt]
    z = np.concatenate(
        [res.results[e]["zs"].transpose(2, 1, 0).reshape(TS, D)
         for e in range(E)], axis=0)
    out = (y + z).reshape(B, S, D)
    return out.astype(x.dtype)



# revision 6
# speedup vs baseline: 1.0042x; 1.0042x over previous
"""MoE (8 routed experts, top-2, + shared expert) on 8 TRN2 NeuronCores.

Strategy: expert-parallel. Host computes the gate (fp32 numpy, exactly
mirroring the reference), gathers each expert's tokens, and core e runs
expert e's SwiGLU (h = silu(x@w1T) * (x@w3T) * cw; y = h_bf16 @ w2T)
over its gathered tokens, plus a 1/8 token-slice of the shared expert.
Host scatters expert outputs back and combines in bf16 expert order.

All tensors fed to the device are pre-arranged on host into
partition-major layouts so every DMA is contiguous per partition:
  activations/weights for matmul lhsT/rhs always have the contraction
  dim chunked as [pi=128, po, free].
"""

import numpy as np
import ml_dtypes

import concourse.mybir as mybir
from concourse import bacc
from concourse.tile import TileContext
from concourse import bass_utils

BF16 = mybir.dt.bfloat16
F32 = mybir.dt.float32

D = 2048          # model dim
I = 1408          # expert inter dim
E = 8             # routed experts
TOPK = 2
N_CORES = 8
DPO = D // 128    # 16 chunks of the model dim
IPO = I // 128    # 11 chunks of the inter dim

_BUILD_CACHE = {}


def _c_blocks(C):
    """Split C columns into equal-ish blocks <= 512, multiples of 8."""
    nb = -(-C // 512)
    per = -(-C // (nb * 8)) * 8
    blocks = []
    off = 0
    while off < C:
        w = min(per, C - off)
        blocks.append((off, w))
        off += w
    return blocks


def _build(C, TS):
    """Build the per-core Bass kernel for routed capacity C and shared
    token-slice TS. Same NEFF runs SPMD on all 8 cores."""
    nc = bacc.Bacc("TRN2", debug=False, enable_asserts=False,
                   num_devices=N_CORES, enable_partition_id=False)

    def din(name, shape, dt=BF16):
        return nc.dram_tensor(name, shape, dt, kind="ExternalInput").ap()

    def dout(name, shape, dt=BF16):
        return nc.dram_tensor(name, shape, dt, kind="ExternalOutput").ap()

    xr = din("xr", [128, DPO, C])            # routed tokens, [d_pi, d_po, c]
    xs = din("xs", [128, DPO, TS])           # shared-expert token slice
    cwb = din("cwb", [128, C], F32)          # combine weights, replicated
    w1t = din("w1t", [IPO, 128, D])          # [i_blk][d_pi][d_po*128+i_c]
    w3t = din("w3t", [IPO, 128, D])
    w2t = din("w2t", [DPO, 128, I])          # [d_blk][i_pi][i_po*128+d_c]
    sw1t = din("sw1t", [IPO, 128, D])
    sw3t = din("sw3t", [IPO, 128, D])
    sw2t = din("sw2t", [DPO, 128, I])
    ye = dout("ye", [128, DPO, C])           # [d_pi, d_po, c]
    zs = dout("zs", [128, DPO, TS])

    Silu = mybir.ActivationFunctionType.Silu

    with TileContext(nc) as tc:
        with tc.tile_pool(name="main", bufs=1) as pool, \
             tc.tile_pool(name="psum", bufs=1, space="PSUM") as pp:
            cw_sb = pool.tile([128, C], F32, tag="cwb", bufs=1, name="cw_sb")

            # HAM prewarm: the PE clock sits at 1.2 GHz until ~3.4us of
            # sustained activity. Burn dummy matmuls on a zeroed tile while
            # the startup DMAs are in flight so the real stream runs warm.
            warm = pool.tile([128, 128], BF16, tag="warm", bufs=1, name="warm")
            nc.gpsimd.memset(warm[:], 0.0)
            wp = pp.tile([128, 128], F32, tag="ps", bufs=8, name="warm_ps")
            for _ in range(26):
                nc.tensor.matmul(wp[:], warm[:], warm[:], start=True, stop=True)

            # routed job first: the second job's startup stream then
            # overlaps the first job's ~113us of phase-B PE work, and the
            # small xs stream doesn't starve routed phase-B weight loads
            jobs = [
                ("r", C, xr, w1t, w3t, w2t, ye, True),
                ("s", TS, xs, sw1t, sw3t, sw2t, zs, False),
            ]
            for jname, CJ, x_d, w1_d, w3_d, w2_d, out_d, scaled in jobs:
                cbs = _c_blocks(CJ)
                x_sb = pool.tile([128, DPO, CJ], BF16, tag=f"x_{jname}",
                                 bufs=1, name=f"x_{jname}")
                # startup: land just enough bytes for the first matmuls
                # (x slice 0 + the first weight chunks) before streaming
                # the rest, so the PE starts ~10us in instead of ~25us
                w13_first = []
                wdr = []
                for wd, wn in ((w1_d, "w1"), (w3_d, "w3")):
                    w_sb = pool.tile([128, DPO, 128], BF16, tag="w13",
                                     bufs=6, name=f"{wn}_{jname}_0")
                    w13_first.append(w_sb)
                    wdr.append(wd[0].rearrange("p (a b) -> p a b", a=DPO))
                # issue order = first-matmul critical path: w1 head chunk,
                # x slice 0, w3 head chunk, then stream the rest
                nc.sync.dma_start(w13_first[0][:, 0:4, :], wdr[0][:, 0:4, :])
                nc.sync.dma_start(x_sb[:, 0, :], x_d[:, 0, :])
                nc.sync.dma_start(w13_first[1][:, 0:4, :], wdr[1][:, 0:4, :])
                nc.sync.dma_start(x_sb[:, 1, :], x_d[:, 1, :])
                for w_sb, wsrc in zip(w13_first, wdr):
                    nc.sync.dma_start(w_sb[:, 4:, :], wsrc[:, 4:, :])
                for dsl in range(2, DPO):
                    nc.sync.dma_start(x_sb[:, dsl, :], x_d[:, dsl, :])
                if scaled:
                    nc.sync.dma_start(cw_sb[:], cwb[:])
                H = pool.tile([128, IPO, CJ], BF16, tag=f"H_{jname}",
                              bufs=1, name=f"H_{jname}")

                # ---- phase A: H = silu(x@w1T) * (x@w3T) [* cw] ----
                for i in range(IPO):
                    if i == 0:
                        w1_sb, w3_sb = w13_first
                    else:
                        w1_sb = pool.tile([128, DPO, 128], BF16, tag="w13",
                                          bufs=6, name=f"w1_{jname}_{i}")
                        nc.sync.dma_start(
                            w1_sb[:],
                            w1_d[i].rearrange("p (a b) -> p a b", a=DPO))
                        w3_sb = pool.tile([128, DPO, 128], BF16, tag="w13",
                                          bufs=6, name=f"w3_{jname}_{i}")
                        nc.sync.dma_start(
                            w3_sb[:],
                            w3_d[i].rearrange("p (a b) -> p a b", a=DPO))
                    p1s = []
                    p3s = []
                    for bi, (off, w) in enumerate(cbs):
                        p1s.append(pp.tile([128, w], F32, tag="ps", bufs=8,
                                           name=f"p1_{jname}_{i}_{bi}"))
                        p3s.append(pp.tile([128, w], F32, tag="ps", bufs=8,
                                           name=f"p3_{jname}_{i}_{bi}"))
                    for d in range(DPO):
                        for bi, (off, w) in enumerate(cbs):
                            nc.tensor.matmul(
                                p1s[bi][:], w1_sb[:, d, :],
                                x_sb[:, d, off:off + w],
                                start=(d == 0), stop=(d == DPO - 1))
                        for bi, (off, w) in enumerate(cbs):
                            nc.tensor.matmul(
                                p3s[bi][:], w3_sb[:, d, :],
                                x_sb[:, d, off:off + w],
                                start=(d == 0), stop=(d == DPO - 1))
                    for bi, (off, w) in enumerate(cbs):
                        s_t = pool.tile([128, w], F32, tag="act1", bufs=6,
                                        name=f"s_{jname}_{i}_{bi}")
                        nc.scalar.activation(s_t[:], p1s[bi][:], Silu)
                        if scaled:
                            t_t = pool.tile([128, w], F32, tag="act2", bufs=6,
                                            name=f"t_{jname}_{i}_{bi}")
                            nc.vector.tensor_mul(t_t[:], p3s[bi][:],
                                                 cw_sb[:, off:off + w])
                            nc.vector.tensor_mul(H[:, i, off:off + w],
                                                 s_t[:], t_t[:])
                        else:
                            nc.vector.tensor_mul(H[:, i, off:off + w],
                                                 s_t[:], p3s[bi][:])

                # ---- phase B: out = H @ w2T ----
                for do in range(DPO):
                    w2_sb = pool.tile([128, IPO, 128], BF16, tag="w2",
                                      bufs=5, name=f"w2_{jname}_{do}")
                    nc.sync.dma_start(
                        w2_sb[:], w2_d[do].rearrange("p (a b) -> p a b", a=IPO))
                    pys = []
                    for bi, (off, w) in enumerate(cbs):
                        pys.append(pp.tile([128, w], F32, tag="ps", bufs=8,
                                           name=f"py_{jname}_{do}_{bi}"))
                    for i in range(IPO):
                        for bi, (off, w) in enumerate(cbs):
                            nc.tensor.matmul(
                                pys[bi][:], w2_sb[:, i, :],
                                H[:, i, off:off + w],
                                start=(i == 0), stop=(i == IPO - 1))
                    y_t = pool.tile([128, CJ], BF16, tag="yo", bufs=4,
                                    name=f"y_{jname}_{do}")
                    for bi, (off, w) in enumerate(cbs):
                        nc.vector.tensor_copy(y_t[:, off:off + w], pys[bi][:])
                    nc.sync.dma_start(out_d[:, do, :], y_t[:])

    nc.finalize()
    return nc


def _get_kernel(C, TS):
    key = (C, TS)
    if key not in _BUILD_CACHE:
        _BUILD_CACHE[key] = _build(C, TS)
    return _BUILD_CACHE[key]


def _pm(a, po):
    """[N, po*128] -> partition-major [128, po, N] contiguous."""
    n = a.shape[0]
    return np.ascontiguousarray(
        a.T.reshape(po, 128, n).transpose(1, 0, 2))


def kernel(x, gate_w, gate_b, w1, w2, w3, sw1, sw2, sw3):
    bf16 = ml_dtypes.bfloat16
    x = np.asarray(x)
    gate_w = np.asarray(gate_w, dtype=np.float32)
    gate_b = np.asarray(gate_b, dtype=np.float32)
    w1 = np.asarray(w1)
    w2 = np.asarray(w2)
    w3 = np.asarray(w3)
    sw1 = np.asarray(sw1)
    sw2 = np.asarray(sw2)
    sw3 = np.asarray(sw3)

    B, S, Dx = x.shape
    assert Dx == D
    T = B * S
    TS = T // N_CORES
    xt = x.reshape(T, D)

    # ---- gate (fp32, mirrors reference: sqrt(softplus), top-2 on biased) ----
    xf = xt.astype(np.float32)
    logits = xf @ gate_w.T
    scores = np.sqrt(np.log1p(np.exp(-np.abs(logits)))
                     + np.maximum(logits, 0.0))
    biased = scores + gate_b
    idx = np.argsort(-biased, axis=1, kind="stable")[:, :TOPK]
    cw = np.zeros((T, E), dtype=np.float32)
    np.put_along_axis(cw, idx, np.take_along_axis(scores, idx, axis=1), axis=1)

    sel = np.zeros((T, E), dtype=bool)
    np.put_along_axis(sel, idx, True, axis=1)
    tok_lists = [np.nonzero(sel[:, e])[0] for e in range(E)]
    counts = np.array([len(t) for t in tok_lists])
    C = max(256, int(-(-counts.max() // 8) * 8))

    nc = _get_kernel(C, TS)

    # ---- per-core input prep ----
    # weight transforms: lhsT layouts, block-major so DMAs are contiguous
    def wA_layout(wm):  # [I, D] -> [IPO, 128, D]; [ib,pi,po*128+ic]
        return np.ascontiguousarray(
            wm.T.reshape(DPO, 128, IPO, 128).transpose(2, 1, 0, 3)
        ).reshape(IPO, 128, D)

    def wB_layout(wm):  # [D, I] -> [DPO, 128, I]; [db,pi,po*128+dc]
        return np.ascontiguousarray(
            wm.T.reshape(IPO, 128, DPO, 128).transpose(2, 1, 0, 3)
        ).reshape(DPO, 128, I)

    sw1t = wA_layout(sw1)
    sw3t = wA_layout(sw3)
    sw2t = wB_layout(sw2)

    in_maps = []
    for e in range(E):
        toks = tok_lists[e]
        cnt = len(toks)
        xg = np.zeros((C, D), dtype=bf16)
        xg[:cnt] = xt[toks]
        cwe = np.zeros((C,), dtype=np.float32)
        cwe[:cnt] = cw[toks, e]
        xs_slice = xt[e * TS:(e + 1) * TS]
        in_maps.append({
            "xr": _pm(xg, DPO),
            "xs": _pm(xs_slice, DPO),
            "cwb": np.ascontiguousarray(
                np.broadcast_to(cwe[None, :], (128, C))),
            "w1t": wA_layout(w1[e]),
            "w3t": wA_layout(w3[e]),
            "w2t": wB_layout(w2[e]),
            "sw1t": sw1t,
            "sw3t": sw3t,
            "sw2t": sw2t,
        })

    res = bass_utils.run_bass_kernel_spmd(
        nc, in_maps, core_ids=list(range(N_CORES)))
    global LAST_RESULT
    LAST_RESULT = res

    # ---- unshard + combine (bf16, reference addition order) ----
    y = np.zeros((T, D), dtype=bf16)
    for e in range(E):
        toks = tok_lists[e]
        cnt = len(toks)
        ye = res.results[e]["ye"]                       # [128, DPO, C]
        ye_tok = ye.transpose(2, 1, 0).reshape(C, D)    # [c, d]
        y[toks] = y[toks] + ye_tok[:cnt]
    z = np.concatenate(
        [res.results[e]["zs"].transpose(2, 1, 0).reshape(TS, D)
         for e in range(E)], axis=0)
    out = (y + z).reshape(B, S, D)
    return out.astype(x.dtype)



# revision 7
# speedup vs baseline: 1.0336x; 1.0292x over previous
"""MoE (8 routed experts, top-2, + shared expert) on 8 TRN2 NeuronCores.

Strategy: pair-split expert parallelism. Host computes the gate (fp32
numpy, exactly mirroring the reference). Cores work in pairs: each pair
of cores covers two experts via two routed token slots per core
(capacities C1 >= C2, fixed at compile time), so an overloaded expert's
tokens can be split across its pair instead of padding every core to
the global max expert count. Every core also runs a 1/8 token-slice of
the shared expert. Host scatters expert outputs back and combines in
bf16 expert order.

Per-core slot layout (same NEFF on all 8 cores):
  slot B (cap C2) -> slot A (cap C1) -> shared slice (TS)
Each slot is a generic SwiGLU job: (weight set, tokens, combine wts).

All tensors fed to the device are pre-arranged on host into
partition-major layouts so every DMA is contiguous per partition:
  activations/weights for matmul lhsT/rhs always have the contraction
  dim chunked as [pi=128, po, free].
"""

import numpy as np
import ml_dtypes

import concourse.mybir as mybir
from concourse import bacc
from concourse.tile import TileContext
from concourse import bass_utils

BF16 = mybir.dt.bfloat16
F32 = mybir.dt.float32

D = 2048          # model dim
I = 1408          # expert inter dim
E = 8             # routed experts
TOPK = 2
N_CORES = 8
DPO = D // 128    # 16 chunks of the model dim
IPO = I // 128    # 11 chunks of the inter dim

_BUILD_CACHE = {}


def _ceil8(x):
    return int(-(-x // 8) * 8)


def _c_blocks(C):
    """Split C columns into equal-ish blocks <= 512, multiples of 8."""
    nb = -(-C // 512)
    per = -(-C // (nb * 8)) * 8
    blocks = []
    off = 0
    while off < C:
        w = min(per, C - off)
        blocks.append((off, w))
        off += w
    return blocks


def _build(C1, C2, TS):
    """Build the per-core Bass kernel: routed slots of capacity C2 and C1
    plus shared token-slice TS. Same NEFF runs SPMD on all 8 cores."""
    nc = bacc.Bacc("TRN2", debug=False, enable_asserts=False,
                   num_devices=N_CORES, enable_partition_id=False)

    def din(name, shape, dt=BF16):
        return nc.dram_tensor(name, shape, dt, kind="ExternalInput").ap()

    def dout(name, shape, dt=BF16):
        return nc.dram_tensor(name, shape, dt, kind="ExternalOutput").ap()

    # job descriptors: (name, cap, scaled). Processing order: the smaller
    # routed slot first (gentler weight-stream ramp during DMA startup),
    # the big slot second (its x/weights prefetch during slot-B compute),
    # shared last.
    jdefs = [("b", C2, True), ("a", C1, True), ("s", TS, False)]
    dram = {}
    for jname, CJ, scaled in jdefs:
        dram[jname] = {
            "x": din(f"x_{jname}", [128, DPO, CJ]),
            "w1": din(f"w1_{jname}", [IPO, 128, D]),
            "w3": din(f"w3_{jname}", [IPO, 128, D]),
            "w2": din(f"w2_{jname}", [DPO, 128, I]),
            "out": dout(f"y_{jname}", [128, DPO, CJ]),
        }
        if scaled:
            dram[jname]["cw"] = din(f"cw_{jname}", [128, CJ], F32)

    Silu = mybir.ActivationFunctionType.Silu

    with TileContext(nc) as tc:
        with tc.tile_pool(name="main", bufs=1) as pool, \
             tc.tile_pool(name="psum", bufs=1, space="PSUM") as pp:
            # HAM prewarm: the PE clock sits at 1.2 GHz until ~3.4us of
            # sustained activity. Burn dummy matmuls on a zeroed tile while
            # the startup DMAs are in flight so the real stream runs warm.
            warm = pool.tile([128, 128], BF16, tag="warm", bufs=1, name="warm")
            nc.gpsimd.memset(warm[:], 0.0)
            wp = pp.tile([128, 128], F32, tag="ps", bufs=8, name="warm_ps")
            for _ in range(26):
                nc.tensor.matmul(wp[:], warm[:], warm[:], start=True, stop=True)

            for jname, CJ, scaled in jdefs:
                dd = dram[jname]
                cbs = _c_blocks(CJ)
                x_sb = pool.tile([128, DPO, CJ], BF16, tag=f"x_{jname}",
                                 bufs=1, name=f"x_{jname}")
                w13_first = []
                wdr = []
                for wkey in ("w1", "w3"):
                    w_sb = pool.tile([128, DPO, 128], BF16, tag="w13",
                                     bufs=8, name=f"{wkey}_{jname}_0")
                    w13_first.append(w_sb)
                    wdr.append(dd[wkey][0].rearrange("p (a b) -> p a b", a=DPO))
                # issue order = first-matmul critical path: w1 head chunk,
                # x slice 0, w3 head chunk, then stream the rest
                nc.sync.dma_start(w13_first[0][:, 0:4, :], wdr[0][:, 0:4, :])
                nc.sync.dma_start(x_sb[:, 0, :], dd["x"][:, 0, :])
                nc.sync.dma_start(w13_first[1][:, 0:4, :], wdr[1][:, 0:4, :])
                nc.sync.dma_start(x_sb[:, 1, :], dd["x"][:, 1, :])
                for w_sb, wsrc in zip(w13_first, wdr):
                    nc.sync.dma_start(w_sb[:, 4:, :], wsrc[:, 4:, :])
                for dsl in range(2, DPO):
                    nc.sync.dma_start(x_sb[:, dsl, :], dd["x"][:, dsl, :])
                if scaled:
                    cw_sb = pool.tile([128, CJ], F32, tag=f"cw_{jname}",
                                      bufs=1, name=f"cw_{jname}")
                    nc.sync.dma_start(cw_sb[:], dd["cw"][:])
                H = pool.tile([128, IPO, CJ], BF16, tag=f"H_{jname}",
                              bufs=1, name=f"H_{jname}")

                # ---- phase A: H = silu(x@w1T) * (x@w3T) [* cw] ----
                for i in range(IPO):
                    if i == 0:
                        w1_sb, w3_sb = w13_first
                    else:
                        w1_sb = pool.tile([128, DPO, 128], BF16, tag="w13",
                                          bufs=8, name=f"w1_{jname}_{i}")
                        nc.sync.dma_start(
                            w1_sb[:],
                            dd["w1"][i].rearrange("p (a b) -> p a b", a=DPO))
                        w3_sb = pool.tile([128, DPO, 128], BF16, tag="w13",
                                          bufs=8, name=f"w3_{jname}_{i}")
                        nc.sync.dma_start(
                            w3_sb[:],
                            dd["w3"][i].rearrange("p (a b) -> p a b", a=DPO))
                    p1s = []
                    p3s = []
                    for bi, (off, w) in enumerate(cbs):
                        p1s.append(pp.tile([128, w], F32, tag="ps", bufs=8,
                                           name=f"p1_{jname}_{i}_{bi}"))
                        p3s.append(pp.tile([128, w], F32, tag="ps", bufs=8,
                                           name=f"p3_{jname}_{i}_{bi}"))
                    for d in range(DPO):
                        for bi, (off, w) in enumerate(cbs):
                            nc.tensor.matmul(
                                p1s[bi][:], w1_sb[:, d, :],
                                x_sb[:, d, off:off + w],
                                start=(d == 0), stop=(d == DPO - 1))
                        for bi, (off, w) in enumerate(cbs):
                            nc.tensor.matmul(
                                p3s[bi][:], w3_sb[:, d, :],
                                x_sb[:, d, off:off + w],
                                start=(d == 0), stop=(d == DPO - 1))
                    for bi, (off, w) in enumerate(cbs):
                        s_t = pool.tile([128, w], F32, tag="act1", bufs=6,
                                        name=f"s_{jname}_{i}_{bi}")
                        nc.scalar.activation(s_t[:], p1s[bi][:], Silu)
                        if scaled:
                            t_t = pool.tile([128, w], F32, tag="act2", bufs=6,
                                            name=f"t_{jname}_{i}_{bi}")
                            nc.vector.tensor_mul(t_t[:], p3s[bi][:],
                                                 cw_sb[:, off:off + w])
                            nc.vector.tensor_mul(H[:, i, off:off + w],
                                                 s_t[:], t_t[:])
                        else:
                            nc.vector.tensor_mul(H[:, i, off:off + w],
                                                 s_t[:], p3s[bi][:])

                # ---- phase B: out = H @ w2T ----
                for do in range(DPO):
                    w2_sb = pool.tile([128, IPO, 128], BF16, tag="w2",
                                      bufs=6, name=f"w2_{jname}_{do}")
                    nc.sync.dma_start(
                        w2_sb[:],
                        dd["w2"][do].rearrange("p (a b) -> p a b", a=IPO))
                    pys = []
                    for bi, (off, w) in enumerate(cbs):
                        pys.append(pp.tile([128, w], F32, tag="ps", bufs=8,
                                           name=f"py_{jname}_{do}_{bi}"))
                    for i in range(IPO):
                        for bi, (off, w) in enumerate(cbs):
                            nc.tensor.matmul(
                                pys[bi][:], w2_sb[:, i, :],
                                H[:, i, off:off + w],
                                start=(i == 0), stop=(i == IPO - 1))
                    y_t = pool.tile([128, CJ], BF16, tag="yo", bufs=4,
                                    name=f"y_{jname}_{do}")
                    for bi, (off, w) in enumerate(cbs):
                        nc.vector.tensor_copy(y_t[:, off:off + w], pys[bi][:])
                    nc.sync.dma_start(dd["out"][:, do, :], y_t[:])

    nc.finalize()
    return nc


def _get_kernel(C1, C2, TS):
    key = (C1, C2, TS)
    if key not in _BUILD_CACHE:
        _BUILD_CACHE[key] = _build(C1, C2, TS)
    return _BUILD_CACHE[key]


def _pm(a, po):
    """[N, po*128] -> partition-major [128, po, N] contiguous."""
    n = a.shape[0]
    return np.ascontiguousarray(
        a.T.reshape(po, 128, n).transpose(1, 0, 2))


def _plan_slots(counts):
    """Pick slot capacities (C1 >= C2) and per-core slot assignments.

    Pairs the i-th largest expert with the i-th smallest. The first k
    pairs split each expert evenly across the pair's two cores (A-mode);
    the rest put one expert per core spanning both slots (B-mode). k is
    chosen to minimize C1 + C2.
    Returns (C1, C2, assign) where assign[core] = [(slot, expert, tok_lo,
    tok_hi), ...] listing which token ranges of which expert fill each
    slot ("slot" in {"a", "b"}).
    """
    order = np.argsort(-counts, kind="stable")
    np_pairs = [(int(order[i]), int(order[E - 1 - i])) for i in range(E // 2)]
    best = None
    for k in range(1, E // 2 + 1):
        A, B = np_pairs[:k], np_pairs[k:]
        c1 = max(_ceil8(-(-counts[a] // 2)) for a, _ in A)
        c2 = max(_ceil8(-(-counts[b] // 2)) for _, b in A)
        S = max(c1 + c2, max((_ceil8(int(counts[a])) for a, _ in B),
                             default=0))
        c2 = S - c1
        ok = all(counts[a] <= 2 * c1 and counts[b] <= 2 * c2 for a, b in A)
        ok &= all(counts[a] <= S and counts[b] <= S for a, b in B)
        if ok and (best is None or S < best[0]):
            best = (S, k, c1, c2)
    S, k, C1, C2 = best
    assign = [[] for _ in range(N_CORES)]
    for p, (a, b) in enumerate(np_pairs):
        c0, c1c = 2 * p, 2 * p + 1
        ca, cb = int(counts[a]), int(counts[b])
        if p < k:  # A-mode: split each expert across the pair
            ha, hb = -(-ca // 2), -(-cb // 2)
            assign[c0].append(("a", a, 0, ha))
            assign[c1c].append(("a", a, ha, ca))
            assign[c0].append(("b", b, 0, hb))
            assign[c1c].append(("b", b, hb, cb))
        else:      # B-mode: one expert per core, spanning both slots
            sa, sb = min(ca, C1), min(cb, C1)
            assign[c0].append(("a", a, 0, sa))
            assign[c0].append(("b", a, sa, ca))
            assign[c1c].append(("a", b, 0, sb))
            assign[c1c].append(("b", b, sb, cb))
    return C1, C2, assign


def kernel(x, gate_w, gate_b, w1, w2, w3, sw1, sw2, sw3):
    bf16 = ml_dtypes.bfloat16
    x = np.asarray(x)
    gate_w = np.asarray(gate_w, dtype=np.float32)
    gate_b = np.asarray(gate_b, dtype=np.float32)
    w1 = np.asarray(w1)
    w2 = np.asarray(w2)
    w3 = np.asarray(w3)
    sw1 = np.asarray(sw1)
    sw2 = np.asarray(sw2)
    sw3 = np.asarray(sw3)

    B, S, Dx = x.shape
    assert Dx == D
    T = B * S
    TS = T // N_CORES
    xt = x.reshape(T, D)

    # ---- gate (fp32, mirrors reference: sqrt(softplus), top-2 on biased) ----
    xf = xt.astype(np.float32)
    logits = xf @ gate_w.T
    scores = np.sqrt(np.log1p(np.exp(-np.abs(logits)))
                     + np.maximum(logits, 0.0))
    biased = scores + gate_b
    idx = np.argsort(-biased, axis=1, kind="stable")[:, :TOPK]
    cw = np.zeros((T, E), dtype=np.float32)
    np.put_along_axis(cw, idx, np.take_along_axis(scores, idx, axis=1), axis=1)

    sel = np.zeros((T, E), dtype=bool)
    np.put_along_axis(sel, idx, True, axis=1)
    tok_lists = [np.nonzero(sel[:, e])[0] for e in range(E)]
    counts = np.array([len(t) for t in tok_lists])

    C1, C2, assign = _plan_slots(counts)
    nc = _get_kernel(C1, C2, TS)

    # ---- per-core input prep ----
    # weight transforms: lhsT layouts, block-major so DMAs are contiguous
    def wA_layout(wm):  # [I, D] -> [IPO, 128, D]; [ib,pi,po*128+ic]
        return np.ascontiguousarray(
            wm.T.reshape(DPO, 128, IPO, 128).transpose(2, 1, 0, 3)
        ).reshape(IPO, 128, D)

    def wB_layout(wm):  # [D, I] -> [DPO, 128, I]; [db,pi,po*128+dc]
        return np.ascontiguousarray(
            wm.T.reshape(IPO, 128, DPO, 128).transpose(2, 1, 0, 3)
        ).reshape(DPO, 128, I)

    w1t = [wA_layout(w1[e]) for e in range(E)]
    w3t = [wA_layout(w3[e]) for e in range(E)]
    w2t = [wB_layout(w2[e]) for e in range(E)]
    sw1t = wA_layout(sw1)
    sw3t = wA_layout(sw3)
    sw2t = wB_layout(sw2)

    caps = {"a": C1, "b": C2}
    zero_w1 = np.zeros((IPO, 128, D), dtype=bf16)
    zero_w2 = np.zeros((DPO, 128, I), dtype=bf16)
    in_maps = []
    # pieces[e] = ordered list of (core, slot, n_tokens) for output combine
    pieces = [[] for _ in range(E)]
    for core in range(N_CORES):
        im = {
            "x_s": _pm(xt[core * TS:(core + 1) * TS], DPO),
            "w1_s": sw1t, "w3_s": sw3t, "w2_s": sw2t,
        }
        filled = set()
        for slot, e, lo, hi in assign[core]:
            Cs = caps[slot]
            cnt = hi - lo
            toks = tok_lists[e][lo:hi]
            xg = np.zeros((Cs, D), dtype=bf16)
            xg[:cnt] = xt[toks]
            cwe = np.zeros((Cs,), dtype=np.float32)
            cwe[:cnt] = cw[toks, e]
            im[f"x_{slot}"] = _pm(xg, DPO)
            im[f"cw_{slot}"] = np.ascontiguousarray(
                np.broadcast_to(cwe[None, :], (128, Cs)))
            im[f"w1_{slot}"] = w1t[e]
            im[f"w3_{slot}"] = w3t[e]
            im[f"w2_{slot}"] = w2t[e]
            pieces[e].append((core, slot, cnt))
            filled.add(slot)
        for slot in ("a", "b"):
            if slot not in filled:
                Cs = caps[slot]
                im[f"x_{slot}"] = np.zeros((128, DPO, Cs), dtype=bf16)
                im[f"cw_{slot}"] = np.zeros((128, Cs), dtype=np.float32)
                im[f"w1_{slot}"] = zero_w1
                im[f"w3_{slot}"] = zero_w1
                im[f"w2_{slot}"] = zero_w2
        in_maps.append(im)

    res = bass_utils.run_bass_kernel_spmd(
        nc, in_maps, core_ids=list(range(N_CORES)))
    global LAST_RESULT
    LAST_RESULT = res

    # ---- unshard + combine (bf16, reference addition order) ----
    y = np.zeros((T, D), dtype=bf16)
    for e in range(E):
        toks = tok_lists[e]
        parts = []
        for core, slot, cnt in pieces[e]:
            ye = res.results[core][f"y_{slot}"]           # [128, DPO, Cs]
            ye_tok = ye.transpose(2, 1, 0).reshape(caps[slot], D)
            parts.append(ye_tok[:cnt])
        ye_all = np.concatenate(parts, axis=0) if len(parts) > 1 else parts[0]
        y[toks] = y[toks] + ye_all
    z = np.concatenate(
        [res.results[core]["y_s"].transpose(2, 1, 0).reshape(TS, D)
         for core in range(N_CORES)], axis=0)
    out = (y + z).reshape(B, S, D)
    return out.astype(x.dtype)
